# revision 14
# baseline (speedup 1.0000x reference)
"""Trainium2 Bass kernel for gnn_message_passing (nn_CMP_67181878444960).

Strategy (8-core SPMD, no collectives):
  - Host converts the edge list into two dense [V, V] count matrices
    (pos / neg).  pooled = A @ feats is then a dense matmul: each core
    computes the pooled features for its 128 nodes by streaming the full
    feats matrix [1024, 16384] through the PE (f32r, K-tiled by 128),
    spilling pooled to a DRAM scratch tensor.
  - The conv encoder is embarrassingly parallel over nodes: each core
    runs 2 residual blocks + final conv/instance-norm/relu for its 128
    nodes.  Convs are 9 shift-tap matmuls (contraction over channels on
    partitions); two nodes are packed per matmul via block-diagonal
    weights (K=96, M=96).  Boundary zero-padding is handled by clipping
    each tap's output window (PSUM has_written gives write-then-
    accumulate semantics).
"""

import functools
import sys

import numpy as np

for _p in ("/opt/trn_rl_repo",):
    if _p not in sys.path:
        sys.path.insert(0, _p)

import concourse.tile as tile  # noqa: E402
from concourse import bacc, bass_utils, mybir  # noqa: E402
from concourse.tile_rust import add_dep_helper  # noqa: E402

F32 = mybir.dt.float32
F32R = mybir.dt.float32r
BF16 = mybir.dt.bfloat16
AF = mybir.ActivationFunctionType

V, C, H = 1024, 16, 32
SP = H * H            # 1024 spatial
PW = H + 2            # padded row width (zero border)
PSP = PW * PW         # padded spatial per channel
CHW = C * SP          # 16384
C3 = 3 * C            # 48 conv channels
NCORES = 8
NPC = V // NCORES     # 128 nodes per core
EPS = 1e-5

# weight-column layout: 5 layers x 9 taps, then the two residual-conv bias rows
_LAYER_COUT = [C3, C3, C3, C3, C]          # 1a, 1b, 2a, 2b, final
_TAP_OFF = []
_off = 0
for _co in _LAYER_COUT:
    _TAP_OFF.append(_off)
    _off += 9 * 2 * _co
_BIAS1B_OFF = _off
_off += 2 * C3
_BIAS2B_OFF = _off
_off += 2 * C3
_ONES_OFF = _off
_off += 512
WCOLS = _off


def _r32(ap):
    return ap.bitcast(F32R)


def _mi(inst):
    return getattr(inst, "ins", inst)


class _SlotGuard:
    """Explicitly order each pool slot's new first-writer after the previous
    occupant's last accessor (belt-and-braces against mis-synced reuse)."""

    def __init__(self):
        self.state = {}

    def begin(self, tag, bufs, writer_insts):
        idx, hist = self.state.setdefault(tag, [0, {}])
        prev = hist.get(idx % bufs)
        if prev is not None:
            for w in writer_insts:
                add_dep_helper(_mi(w), _mi(prev), True, "slot-reuse guard")

    def end(self, tag, bufs, last_inst):
        st = self.state.setdefault(tag, [0, {}])
        st[1][st[0] % bufs] = last_inst
        st[0] += 1


def build_kernel(tc, aps, npc, v, kt, pass_ranges):
    """Emit the per-core program. aps: dict of dram APs.

    kt: number of 128-row K-tiles in the compacted source pool.
    pass_ranges: [(k0, k1), (k0, k1)] K-tile ranges for the pos / neg
    pooled accumulations (source rows laid out [pos-only|shared|neg-only]).
    """
    nc = tc.nc
    n_chunk = 512            # pooling column chunk
    nchunks = CHW // n_chunk
    npairs = npc // 2

    feats_pool = aps["feats_pool"]
    feats_shard = aps["feats_shard"]
    a_lhsT = aps["a_lhsT"]
    wconv = aps["wconv"]
    biases = aps["biases"]
    out = aps["out"]

    guard = _SlotGuard()
    ctx = {"guard": guard}
    build_kernel._ctx = ctx

    with (
        tc.tile_pool(name="persist", bufs=1) as persist,
        tc.tile_pool(name="psum", bufs=8, space="PSUM") as psum_pool,
    ):
        # ---- persistent SBUF state ----
        wsb = persist.tile([C3 * 2, WCOLS], BF16, tag="wsb")
        pooled = persist.tile([128, 2 * CHW], BF16, tag="pooled")
        bias_sb = persist.tile([128, 6], F32, tag="bias_sb")
        jt = persist.tile([1, 8], F32, tag="jt")
        ctx["wsb"] = wsb
        ctx["bias_sb"] = bias_sb

        # weights/biases go on the gpsimd queue so the feats stream (sync
        # + vector queues) starts immediately; they are only needed by the
        # conv stage ~60us in.
        nc.gpsimd.dma_start(wsb[:], wconv[:, :])
        nc.gpsimd.dma_start(bias_sb[:], biases[:, :])

        # ================= stage 1: pooling =================
        with (
            tc.tile_pool(name="asb", bufs=1) as asb_pool,
            tc.tile_pool(name="fstage", bufs=6) as fstage,
        ):
            a_sb = asb_pool.tile([128, kt * 2 * npc], BF16)
            nc.sync.dma_start(a_sb[:], a_lhsT[:, :])
            for cc in range(nchunks):
                fs = fstage.tile([128, kt * n_chunk], BF16, tag="fs")
                q = nc.sync if cc % 2 == 0 else nc.scalar
                d = q.dma_start(
                    fs[:], feats_pool[cc * 128:(cc + 1) * 128, :])
                guard.begin("fs", 6, [d])
                last_mm = None
                fs_r = fs[:].rearrange("p (k n) -> p k n", k=kt)
                a_r = a_sb[:].rearrange("p (k m) -> p k m", k=kt)
                for m in range(2):
                    k0, k1 = pass_ranges[m]
                    pp = psum_pool.tile([128, n_chunk], F32, tag="ps")
                    for k in range(k0, k1):
                        last_mm = nc.tensor.matmul(
                            pp[:npc, :],
                            a_r[:, k, m * npc:(m + 1) * npc],
                            fs_r[:, k, :],
                            start=(k == k0),
                            stop=(k == k1 - 1),
                        )
                    nc.vector.tensor_copy(
                        pooled[:npc, m * CHW + cc * n_chunk:
                               m * CHW + (cc + 1) * n_chunk],
                        pp[:npc, :],
                    )
                guard.end("fs", 6, last_mm)

        # ================= stage 2: conv encoder =================
        with (
            tc.tile_pool(name="xt", bufs=5) as xpool,
            tc.tile_pool(name="ht", bufs=5) as hpool,
            tc.tile_pool(name="ot", bufs=3) as opool,
            tc.tile_pool(name="nrm", bufs=6) as nrm,
        ):
            # prefetch the feats channels of the first pairs' x tiles on the
            # sync queue; they only depend on DRAM and overlap pooling
            npf = min(4, npairs)
            pre = {}
            for p in range(npf):
                x = xpool.tile([2 * C3, SP], BF16, tag="x")
                wrts = [
                    nc.sync.dma_start(
                        x[48 * n:48 * n + 16, :],
                        feats_shard[2 * p + n:2 * p + n + 1, :].rearrange(
                            "o (c s) -> (o c) s", c=C))
                    for n in range(2)
                ]
                pre[p] = (x, wrts)

            for p in range(npairs):
                # x = [feats | pooled_pos | pooled_neg] per node, unpadded
                if p in pre:
                    x, wrts = pre[p]
                else:
                    x = xpool.tile([2 * C3, SP], BF16, tag="x")
                    wrts = [
                        nc.gpsimd.dma_start(
                            x[48 * n:48 * n + 16, :],
                            feats_shard[2 * p + n:2 * p + n + 1, :].rearrange(
                                "o (c s) -> (o c) s", c=C))
                        for n in range(2)
                    ]
                for n in range(2):
                    for m in range(2):
                        wrts.append(nc.gpsimd.dma_start(
                            x[48 * n + 16 * (m + 1):48 * n + 16 * (m + 2), :],
                            pooled[2 * p + n:2 * p + n + 1,
                                   m * CHW:(m + 1) * CHW],
                        ))
                guard.begin("x", 5, wrts)

                # residual block 1
                h = hpool.tile([2 * C3, SP], BF16, tag="h")
                hw = _conv(tc, psum_pool, x, h, 0, relu=True, bias_col=0)
                guard.begin("h", 5, hw)
                hl = _conv(tc, psum_pool, h, x, 1, resid=True, bias_col=4)
                guard.end("h", 5, hl[-1])
                # residual block 2
                h2 = hpool.tile([2 * C3, SP], BF16, tag="h")
                hw2 = _conv(tc, psum_pool, x, h2, 2, relu=True, bias_col=1)
                guard.begin("h", 5, hw2)
                hl2 = _conv(tc, psum_pool, h2, x, 3, resid=True, bias_col=5)
                guard.end("h", 5, hl2[-1])
                # final conv + instance norm + relu
                ot = opool.tile([2 * C, SP], F32, tag="ot")
                ow = _conv(tc, psum_pool, x, ot, 4, final=True, bias_col=2)
                guard.begin("ot", 3, ow)
                guard.end("x", 5, ow[-1])

                stats = nrm.tile([2 * C, 12], F32, tag="stats")
                mv = nrm.tile([2 * C, 2], F32, tag="mv")
                sc = nrm.tile([2 * C, 3], F32, tag="sc")
                nc.vector.bn_stats(stats[:, 0:6], ot[:, 0:512])
                nc.vector.bn_stats(stats[:, 6:12], ot[:, 512:1024])
                nc.vector.bn_aggr(mv[:], stats[:])
                # sc0 = sqrt(var+eps); sc1 = 1/sc0; sc2 = -mean/sc0
                nc.scalar.activation(sc[:, 0:1], mv[:, 1:2], AF.Sqrt,
                                     bias=bias_sb[:2 * C, 3:4])
                nc.vector.reciprocal(sc[:, 1:2], sc[:, 0:1])
                nc.vector.tensor_scalar(
                    sc[:, 2:3], mv[:, 0:1], sc[:, 1:2], -1.0,
                    op0=mybir.AluOpType.mult, op1=mybir.AluOpType.mult,
                )
                fin = opool.tile([2 * C, SP], F32, tag="fin")
                ap_i = nc.scalar.activation(
                    fin[:], ot[:], AF.Relu, bias=sc[:, 2:3], scale=sc[:, 1:2]
                )
                guard.begin("fin", 3, [ap_i])
                guard.end("ot", 3, ap_i)
                od = nc.sync.dma_start(out[2 * p:2 * p + 2, :], fin[:])
                guard.end("fin", 3, od)


def _conv(tc, psum_pool, xin, xout, layer, relu=False, resid=False,
          final=False, bias_col=None):
    """One 3x3 'SAME' conv for a node pair (dense, unpadded tiles).

    xin:  [96, 1024] (node, ch) x spatial
    xout: relu  -> write relu(conv+bias) into xout
          resid -> xout += conv + bias (one fused DVE op per halftile)
          final -> copy conv+bias into xout (2*C partitions)
    Taps are outermost so each weight block is loaded once per pair-layer;
    tap windows are clipped to in-bounds pixels (border taps contribute
    zero), which both trims PE rows and removes the need for padding.
    Returns the per-halftile tail instructions (ACT/DVE).
    """
    nc = tc.nc
    ctx = build_kernel._ctx
    wsb, bias_sb = ctx["wsb"], ctx["bias_sb"]

    cout = _LAYER_COUT[layer]
    m = 2 * cout
    xr = xin[:].rearrange("p (r c) -> p r c", c=H)

    # center tap first: it covers the full halftile, so the start=True
    # matmul initializes every PSUM byte before the clipped taps accumulate
    taps = [(0, 0)] + [(dy, dx) for dy in (-1, 0, 1) for dx in (-1, 0, 1)
                       if (dy, dx) != (0, 0)]

    pp0 = psum_pool.tile([128, 512], F32, tag="ps")
    pp1 = psum_pool.tile([128, 512], F32, tag="ps")
    pps = [pp0, pp1]
    pprs = [pp[:].rearrange("p (r c) -> p r c", c=H) for pp in pps]
    for i, (dy, dx) in enumerate(taps):
        ky, kx = dy + 1, dx + 1
        woff = _TAP_OFF[layer] + (ky * 3 + kx) * m
        lw = wsb[0:2 * C3, woff:woff + m]
        cs, ce = max(0, -dx), H - max(0, dx)
        for nt in range(2):
            r0 = nt * 16
            s = max(r0, -dy)
            e = min(r0 + 16, H - max(0, dy))
            nc.tensor.matmul(
                pprs[nt][:m, s - r0:e - r0, cs:ce],
                lw,
                xr[0:2 * C3, s + dy:e + dy, cs + dx:ce + dx],
                start=(i == 0), stop=(i == len(taps) - 1),
                skip_group_check=True,
            )

    tails = []
    for nt in range(2):
        pp = pps[nt]
        half = slice(nt * 512, (nt + 1) * 512)
        if relu:
            t = nc.scalar.activation(
                xout[:m, half], pp[:m, :],
                AF.Relu, bias=bias_sb[:m, bias_col:bias_col + 1],
            )
        elif final:
            if nt == 0:
                t = nc.scalar.activation(
                    xout[:m, half], pp[:m, :],
                    AF.Identity, bias=bias_sb[:m, bias_col:bias_col + 1],
                )
            else:
                # balance the per-pair tail across ACT and DVE
                t = nc.vector.tensor_scalar(
                    xout[:m, half], pp[:m, :],
                    bias_sb[:m, bias_col:bias_col + 1], 1.0,
                    op0=mybir.AluOpType.add, op1=mybir.AluOpType.mult,
                )
        else:  # resid: xout_half = (psum + bias) + xout_half, one DVE op
            t = nc.vector.scalar_tensor_tensor(
                xout[:m, half], pp[:m, :],
                bias_sb[:m, bias_col:bias_col + 1], xout[:m, half],
                op0=mybir.AluOpType.add, op1=mybir.AluOpType.add,
            )
        tails.append(t)
    return tails


# ======================= host side =======================

def _prep_weights(w_list, b_list):
    """Pack conv weights into the [96, WCOLS] f32 lhsT array."""
    wsb = np.zeros((2 * C3, WCOLS), np.float32)
    for layer, (w, b) in enumerate(zip(w_list, b_list)):
        co = _LAYER_COUT[layer]
        for ky in range(3):
            for kx in range(3):
                lt = np.ascontiguousarray(w[:, :, ky, kx].T)  # [C_in, C_out]
                off = _TAP_OFF[layer] + (ky * 3 + kx) * 2 * co
                wsb[0:C3, off:off + co] = lt
                wsb[C3:2 * C3, off + co:off + 2 * co] = lt
    # residual-conv biases live on partition 0 as K=1 lhsT rows
    wsb[0, _BIAS1B_OFF:_BIAS1B_OFF + 2 * C3] = np.tile(b_list[1], 2)
    wsb[0, _BIAS2B_OFF:_BIAS2B_OFF + 2 * C3] = np.tile(b_list[3], 2)
    wsb[0, _ONES_OFF:_ONES_OFF + 512] = 1.0
    import ml_dtypes
    return wsb.astype(ml_dtypes.bfloat16)


def _prep_biases(b1a, b2a, bf, b1b, b2b):
    bias = np.zeros((128, 6), np.float32)
    bias[0:96, 0] = np.tile(b1a, 2)
    bias[0:96, 1] = np.tile(b2a, 2)
    bias[0:2 * C, 2] = np.tile(bf, 2)
    bias[:, 3] = EPS
    bias[0:96, 4] = np.tile(b1b, 2)
    bias[0:96, 5] = np.tile(b2b, 2)
    return bias


def _build_compact_pool(edges, v, ncores, npc):
    """Per-core compaction of pooling sources.

    For each core, the union of edge sources feeding its nodes is laid
    out [pos-only | shared | neg-only] and padded to kt*128 rows.  The
    pos accumulation then only needs K-tiles [0, pos_k1), the neg one
    [neg_k0, kt).  Returns (kt, pass_ranges, per-core list of
    (src_rows, a_compact [2*npc, kt*128])).
    """
    src, lab, dst = edges[:, 0], edges[:, 1], edges[:, 2]
    per_core = []
    for c in range(ncores):
        lo, hi = c * npc, (c + 1) * npc
        # (dst_local, src_global) contribution pairs per sign
        pairs = []
        for mask in (lab > 0, lab < 0):
            s, d = src[mask], dst[mask]
            sel1 = (d >= lo) & (d < hi)
            sel2 = (s >= lo) & (s < hi)
            pairs.append((np.concatenate([d[sel1] - lo, s[sel2] - lo]),
                          np.concatenate([s[sel1], d[sel2]])))
        pos_srcs = set(pairs[0][1].tolist())
        neg_srcs = set(pairs[1][1].tolist())
        shared = pos_srcs & neg_srcs
        order = (sorted(pos_srcs - shared) + sorted(shared)
                 + sorted(neg_srcs - shared))
        per_core.append((pairs, len(pos_srcs - shared), len(shared),
                         len(neg_srcs - shared), order))

    kt = max(int(np.ceil((p + s + n) / 128)) for _, p, s, n, _ in per_core)
    pos_k1 = max(int(np.ceil((p + s) / 128)) for _, p, s, _, _ in per_core)
    neg_k0 = min(p // 128 for _, p, _, _, _ in per_core)
    pass_ranges = ((0, pos_k1), (neg_k0, kt))

    out = []
    for pairs, p, s, n, order in per_core:
        colof = {g: i for i, g in enumerate(order)}
        a = np.zeros((2 * npc, kt * 128), np.float32)
        for mi, (dl, sg) in enumerate(pairs):
            np.add.at(a, (mi * npc + dl, [colof[g] for g in sg]), 1.0)
        rows = np.zeros(kt * 128, np.int64)
        rows[:len(order)] = order
        out.append((rows, a))
    return kt, pass_ranges, out


@functools.lru_cache(maxsize=2)
def _build_module(npc, v, ncores, kt, pass_ranges):
    nc = bacc.Bacc(
        "TRN2", target_bir_lowering=False, debug=False,
        enable_asserts=False, num_devices=ncores,
    )
    aps = {
        "feats_pool": nc.dram_tensor("feats_pool", [(CHW // 512) * 128,
                                     kt * 512], BF16,
                                     kind="ExternalInput").ap(),
        "feats_shard": nc.dram_tensor("feats_shard", [npc, CHW], BF16,
                                      kind="ExternalInput").ap(),
        "a_lhsT": nc.dram_tensor("a_lhsT", [128, kt * 2 * npc], BF16,
                                 kind="ExternalInput").ap(),
        "wconv": nc.dram_tensor("wconv", [2 * C3, WCOLS], BF16,
                                kind="ExternalInput").ap(),
        "biases": nc.dram_tensor("biases", [128, 6], F32,
                                 kind="ExternalInput").ap(),
        "out": nc.dram_tensor("out", [npc, CHW], F32,
                              kind="ExternalOutput").ap(),
    }
    with tile.TileContext(nc) as tc:
        build_kernel(tc, aps, npc, v, kt, pass_ranges)
    nc.compile()
    return nc


def make_in_maps(feats, edges, w1a, b1a, w1b, b1b, w2a, b2a, w2b, b2b,
                 wf, bf, ncores=NCORES, v=V):
    import ml_dtypes
    feats = np.ascontiguousarray(np.asarray(feats, np.float32)).reshape(v, CHW)
    edges = np.asarray(edges)
    npc = v // ncores
    kt, pass_ranges, compact = _build_compact_pool(edges, v, ncores, npc)
    wsb = _prep_weights(
        [np.asarray(w) for w in (w1a, w1b, w2a, w2b, wf)],
        [np.asarray(b) for b in (b1a, b1b, b2a, b2b, bf)],
    )
    bias = _prep_biases(np.asarray(b1a), np.asarray(b2a), np.asarray(bf),
                    np.asarray(b1b), np.asarray(b2b))
    feats_bf = feats.astype(ml_dtypes.bfloat16)
    nch = CHW // 512
    in_maps = []
    for i in range(ncores):
        rows = slice(i * npc, (i + 1) * npc)
        src_rows, a_cmp = compact[i]
        fsel = feats_bf[src_rows]                      # [kt*128, CHW]
        fp = fsel.reshape(kt, 128, nch, 512).transpose(2, 1, 0, 3)
        fp = np.ascontiguousarray(fp).reshape(nch * 128, kt * 512)
        alt = a_cmp.T.reshape(kt, 128, 2 * npc).transpose(1, 0, 2)
        alt = np.ascontiguousarray(alt).reshape(128, kt * 2 * npc)
        in_maps.append({
            "feats_pool": fp,
            "feats_shard": np.ascontiguousarray(feats_bf[rows]),
            "a_lhsT": alt.astype(ml_dtypes.bfloat16),
            "wconv": wsb,
            "biases": bias,
        })
    return in_maps, kt, pass_ranges


def run(inputs, trace=False):
    in_maps, kt, pass_ranges = make_in_maps(**inputs)
    nc = _build_module(NPC, V, NCORES, kt, pass_ranges)
    res = bass_utils.run_bass_kernel_spmd(
        nc, in_maps, core_ids=list(range(NCORES)), trace=trace,
    )
    out = np.concatenate(
        [res.results[i]["out"] for i in range(NCORES)], axis=0
    ).reshape(V, C, H, H)
    return out, res


def kernel(**inputs):
    out, _ = run(inputs, trace=False)
    return out



# revision 16
# speedup vs baseline: 1.1522x; 1.1522x over previous
"""Trainium2 Bass kernel for gnn_message_passing (nn_CMP_67181878444960).

Strategy (8-core SPMD, no collectives):
  - Host converts the edge list into two dense [V, V] count matrices
    (pos / neg).  pooled = A @ feats is then a dense matmul: each core
    computes the pooled features for its 128 nodes by streaming the full
    feats matrix [1024, 16384] through the PE (f32r, K-tiled by 128),
    spilling pooled to a DRAM scratch tensor.
  - The conv encoder is embarrassingly parallel over nodes: each core
    runs 2 residual blocks + final conv/instance-norm/relu for its 128
    nodes.  Convs are 9 shift-tap matmuls (contraction over channels on
    partitions); two nodes are packed per matmul via block-diagonal
    weights (K=96, M=96).  Boundary zero-padding is handled by clipping
    each tap's output window (PSUM has_written gives write-then-
    accumulate semantics).
"""

import functools
import sys

import numpy as np

for _p in ("/opt/trn_rl_repo",):
    if _p not in sys.path:
        sys.path.insert(0, _p)

import concourse.tile as tile  # noqa: E402
from concourse import bacc, bass_utils, mybir  # noqa: E402
from concourse.tile_rust import add_dep_helper  # noqa: E402

F32 = mybir.dt.float32
F32R = mybir.dt.float32r
BF16 = mybir.dt.bfloat16
AF = mybir.ActivationFunctionType

V, C, H = 1024, 16, 32
SP = H * H            # 1024 spatial
PW = H + 2            # padded row width (zero border)
PSP = PW * PW         # padded spatial per channel
CHW = C * SP          # 16384
C3 = 3 * C            # 48 conv channels
NCORES = 8
NPC = V // NCORES     # 128 nodes per core
EPS = 1e-5

# weight-column layout: 5 layers x 9 taps, then the two residual-conv bias rows
_LAYER_COUT = [C3, C3, C3, C3, C]          # 1a, 1b, 2a, 2b, final
_TAP_OFF = []
_off = 0
for _co in _LAYER_COUT:
    _TAP_OFF.append(_off)
    _off += 9 * 2 * _co
_BIAS1B_OFF = _off
_off += 2 * C3
_BIAS2B_OFF = _off
_off += 2 * C3
_ONES_OFF = _off
_off += 512
WCOLS = _off


def _r32(ap):
    return ap.bitcast(F32R)


def _mi(inst):
    return getattr(inst, "ins", inst)


class _SlotGuard:
    """Explicitly order each pool slot's new first-writer after the previous
    occupant's last accessor (belt-and-braces against mis-synced reuse)."""

    def __init__(self):
        self.state = {}

    def begin(self, tag, bufs, writer_insts):
        idx, hist = self.state.setdefault(tag, [0, {}])
        prev = hist.get(idx % bufs)
        if prev is not None:
            for w in writer_insts:
                add_dep_helper(_mi(w), _mi(prev), True, "slot-reuse guard")

    def end(self, tag, bufs, last_inst):
        st = self.state.setdefault(tag, [0, {}])
        st[1][st[0] % bufs] = last_inst
        st[0] += 1


def build_kernel(tc, aps, npc, v, kt, pass_ranges):
    """Emit the per-core program. aps: dict of dram APs.

    kt: number of 128-row K-tiles in the compacted source pool.
    pass_ranges: [(k0, k1), (k0, k1)] K-tile ranges for the pos / neg
    pooled accumulations (source rows laid out [pos-only|shared|neg-only]).
    """
    nc = tc.nc
    n_chunk = 512            # pooling column chunk
    nchunks = CHW // n_chunk
    npairs = npc // 2

    feats_pool = aps["feats_pool"]
    feats_shard = aps["feats_shard"]
    a_lhsT = aps["a_lhsT"]
    wconv = aps["wconv"]
    biases = aps["biases"]
    out = aps["out"]

    guard = _SlotGuard()
    ctx = {"guard": guard}
    build_kernel._ctx = ctx

    with (
        tc.tile_pool(name="persist", bufs=1) as persist,
        tc.tile_pool(name="psum", bufs=8, space="PSUM") as psum_pool,
    ):
        # ---- persistent SBUF state ----
        wsb = persist.tile([C3 * 2, WCOLS], BF16, tag="wsb")
        pooled = persist.tile([128, 2 * CHW], BF16, tag="pooled")
        bias_sb = persist.tile([128, 6], F32, tag="bias_sb")
        jt = persist.tile([1, 8], F32, tag="jt")
        ctx["wsb"] = wsb
        ctx["bias_sb"] = bias_sb

        # weights/biases go on the gpsimd queue so the feats stream (sync
        # + vector queues) starts immediately; they are only needed by the
        # conv stage ~60us in.
        nc.gpsimd.dma_start(wsb[:], wconv[:, :])
        nc.gpsimd.dma_start(bias_sb[:], biases[:, :])

        # ================= stage 1: pooling =================
        with (
            tc.tile_pool(name="asb", bufs=1) as asb_pool,
            tc.tile_pool(name="fstage", bufs=6) as fstage,
        ):
            a_sb = asb_pool.tile([128, kt * 2 * npc], BF16)
            nc.sync.dma_start(a_sb[:], a_lhsT[:, :])
            for cc in range(nchunks):
                fs = fstage.tile([128, kt * n_chunk], BF16, tag="fs")
                q = nc.sync if cc % 2 == 0 else nc.scalar
                d = q.dma_start(
                    fs[:], feats_pool[cc * 128:(cc + 1) * 128, :])
                guard.begin("fs", 6, [d])
                last_mm = None
                fs_r = fs[:].rearrange("p (k n) -> p k n", k=kt)
                a_r = a_sb[:].rearrange("p (k m) -> p k m", k=kt)
                for m in range(2):
                    k0, k1 = pass_ranges[m]
                    pp = psum_pool.tile([128, n_chunk], F32, tag="ps")
                    for k in range(k0, k1):
                        last_mm = nc.tensor.matmul(
                            pp[:npc, :],
                            a_r[:, k, m * npc:(m + 1) * npc],
                            fs_r[:, k, :],
                            start=(k == k0),
                            stop=(k == k1 - 1),
                        )
                    nc.vector.tensor_copy(
                        pooled[:npc, m * CHW + cc * n_chunk:
                               m * CHW + (cc + 1) * n_chunk],
                        pp[:npc, :],
                    )
                guard.end("fs", 6, last_mm)

        # ================= stage 2: conv encoder =================
        with (
            tc.tile_pool(name="xt", bufs=5) as xpool,
            tc.tile_pool(name="ht", bufs=5) as hpool,
            tc.tile_pool(name="ot", bufs=3) as opool,
            tc.tile_pool(name="nrm", bufs=6) as nrm,
        ):
            # prefetch the feats channels of the first pairs' x tiles on the
            # sync queue; they only depend on DRAM and overlap pooling
            npf = min(4, npairs)
            pre = {}
            for p in range(npf):
                x = xpool.tile([2 * C3, SP], BF16, tag="x")
                wrts = [
                    nc.sync.dma_start(
                        x[48 * n:48 * n + 16, :],
                        feats_shard[2 * p + n:2 * p + n + 1, :].rearrange(
                            "o (c s) -> (o c) s", c=C))
                    for n in range(2)
                ]
                pre[p] = (x, wrts)

            for p in range(npairs):
                # x = [feats | pooled_pos | pooled_neg] per node, unpadded
                if p in pre:
                    x, wrts = pre[p]
                else:
                    x = xpool.tile([2 * C3, SP], BF16, tag="x")
                    wrts = [
                        nc.gpsimd.dma_start(
                            x[48 * n:48 * n + 16, :],
                            feats_shard[2 * p + n:2 * p + n + 1, :].rearrange(
                                "o (c s) -> (o c) s", c=C))
                        for n in range(2)
                    ]
                for n in range(2):
                    for m in range(2):
                        wrts.append(nc.gpsimd.dma_start(
                            x[48 * n + 16 * (m + 1):48 * n + 16 * (m + 2), :],
                            pooled[2 * p + n:2 * p + n + 1,
                                   m * CHW:(m + 1) * CHW],
                        ))
                guard.begin("x", 5, wrts)

                # residual block 1
                h = hpool.tile([2 * C3, SP], BF16, tag="h")
                hw = _conv(tc, psum_pool, x, h, 0, relu=True, bias_col=0)
                guard.begin("h", 5, hw)
                hl = _conv(tc, psum_pool, h, x, 1, resid=True, bias_col=4)
                guard.end("h", 5, hl[-1])
                # residual block 2
                h2 = hpool.tile([2 * C3, SP], BF16, tag="h")
                hw2 = _conv(tc, psum_pool, x, h2, 2, relu=True, bias_col=1)
                guard.begin("h", 5, hw2)
                hl2 = _conv(tc, psum_pool, h2, x, 3, resid=True, bias_col=5)
                guard.end("h", 5, hl2[-1])
                # final conv + instance norm + relu
                ot = opool.tile([2 * C, SP], F32, tag="ot")
                ow = _conv(tc, psum_pool, x, ot, 4, final=True, bias_col=2)
                guard.begin("ot", 3, ow)
                guard.end("x", 5, ow[-1])

                stats = nrm.tile([2 * C, 12], F32, tag="stats")
                mv = nrm.tile([2 * C, 2], F32, tag="mv")
                sc = nrm.tile([2 * C, 3], F32, tag="sc")
                nc.vector.bn_stats(stats[:, 0:6], ot[:, 0:512])
                nc.vector.bn_stats(stats[:, 6:12], ot[:, 512:1024])
                nc.vector.bn_aggr(mv[:], stats[:])
                # sc0 = sqrt(var+eps); sc1 = 1/sc0; sc2 = -mean/sc0
                nc.scalar.activation(sc[:, 0:1], mv[:, 1:2], AF.Sqrt,
                                     bias=bias_sb[:2 * C, 3:4])
                nc.vector.reciprocal(sc[:, 1:2], sc[:, 0:1])
                nc.vector.tensor_scalar(
                    sc[:, 2:3], mv[:, 0:1], sc[:, 1:2], -1.0,
                    op0=mybir.AluOpType.mult, op1=mybir.AluOpType.mult,
                )
                fin = opool.tile([2 * C, SP], F32, tag="fin")
                ap_i = nc.scalar.activation(
                    fin[:], ot[:], AF.Relu, bias=sc[:, 2:3], scale=sc[:, 1:2]
                )
                guard.begin("fin", 3, [ap_i])
                guard.end("ot", 3, ap_i)
                od = nc.sync.dma_start(out[2 * p:2 * p + 2, :], fin[:])
                guard.end("fin", 3, od)


def _conv(tc, psum_pool, xin, xout, layer, relu=False, resid=False,
          final=False, bias_col=None):
    """One 3x3 'SAME' conv for a node pair (dense, unpadded tiles).

    xin:  [96, 1024] (node, ch) x spatial
    xout: relu  -> write relu(conv+bias) into xout
          resid -> xout += conv + bias (one fused DVE op per halftile)
          final -> copy conv+bias into xout (2*C partitions)
    Tap windows are clipped to in-bounds pixels (border taps contribute
    zero), which both trims PE rows and removes the need for padding.
    Returns the per-halftile tail instructions (ACT/DVE).
    """
    nc = tc.nc
    ctx = build_kernel._ctx
    wsb, bias_sb = ctx["wsb"], ctx["bias_sb"]

    cout = _LAYER_COUT[layer]
    m = 2 * cout
    xr = xin[:].rearrange("p (r c) -> p r c", c=H)

    # center tap first: it covers the full halftile, so the start=True
    # matmul initializes every PSUM byte before the clipped taps accumulate
    taps = [(0, 0)] + [(dy, dx) for dy in (-1, 0, 1) for dx in (-1, 0, 1)
                       if (dy, dx) != (0, 0)]

    tails = []
    for nt in range(2):
        r0 = nt * 16
        pp = psum_pool.tile([128, 512], F32, tag="ps")
        ppr = pp[:].rearrange("p (r c) -> p r c", c=H)
        for i, (dy, dx) in enumerate(taps):
            ky, kx = dy + 1, dx + 1
            woff = _TAP_OFF[layer] + (ky * 3 + kx) * m
            cs, ce = max(0, -dx), H - max(0, dx)
            s = max(r0, -dy)
            e = min(r0 + 16, H - max(0, dy))
            nc.tensor.matmul(
                ppr[:m, s - r0:e - r0, cs:ce],
                wsb[0:2 * C3, woff:woff + m],
                xr[0:2 * C3, s + dy:e + dy, cs + dx:ce + dx],
                start=(i == 0), stop=(i == len(taps) - 1),
                skip_group_check=True,
            )

        half = slice(nt * 512, (nt + 1) * 512)
        if relu:
            t = nc.scalar.activation(
                xout[:m, half], pp[:m, :],
                AF.Relu, bias=bias_sb[:m, bias_col:bias_col + 1],
            )
        elif final:
            if nt == 0:
                t = nc.scalar.activation(
                    xout[:m, half], pp[:m, :],
                    AF.Identity, bias=bias_sb[:m, bias_col:bias_col + 1],
                )
            else:
                # balance the per-pair tail across ACT and DVE
                t = nc.vector.tensor_scalar(
                    xout[:m, half], pp[:m, :],
                    bias_sb[:m, bias_col:bias_col + 1], 1.0,
                    op0=mybir.AluOpType.add, op1=mybir.AluOpType.mult,
                )
        else:  # resid: xout_half = (psum + bias) + xout_half, one DVE op
            t = nc.vector.scalar_tensor_tensor(
                xout[:m, half], pp[:m, :],
                bias_sb[:m, bias_col:bias_col + 1], xout[:m, half],
                op0=mybir.AluOpType.add, op1=mybir.AluOpType.add,
            )
        tails.append(t)
    return tails


# ======================= host side =======================

def _prep_weights(w_list, b_list):
    """Pack conv weights into the [96, WCOLS] f32 lhsT array."""
    wsb = np.zeros((2 * C3, WCOLS), np.float32)
    for layer, (w, b) in enumerate(zip(w_list, b_list)):
        co = _LAYER_COUT[layer]
        for ky in range(3):
            for kx in range(3):
                lt = np.ascontiguousarray(w[:, :, ky, kx].T)  # [C_in, C_out]
                off = _TAP_OFF[layer] + (ky * 3 + kx) * 2 * co
                wsb[0:C3, off:off + co] = lt
                wsb[C3:2 * C3, off + co:off + 2 * co] = lt
    # residual-conv biases live on partition 0 as K=1 lhsT rows
    wsb[0, _BIAS1B_OFF:_BIAS1B_OFF + 2 * C3] = np.tile(b_list[1], 2)
    wsb[0, _BIAS2B_OFF:_BIAS2B_OFF + 2 * C3] = np.tile(b_list[3], 2)
    wsb[0, _ONES_OFF:_ONES_OFF + 512] = 1.0
    import ml_dtypes
    return wsb.astype(ml_dtypes.bfloat16)


def _prep_biases(b1a, b2a, bf, b1b, b2b):
    bias = np.zeros((128, 6), np.float32)
    bias[0:96, 0] = np.tile(b1a, 2)
    bias[0:96, 1] = np.tile(b2a, 2)
    bias[0:2 * C, 2] = np.tile(bf, 2)
    bias[:, 3] = EPS
    bias[0:96, 4] = np.tile(b1b, 2)
    bias[0:96, 5] = np.tile(b2b, 2)
    return bias


def _build_compact_pool(edges, v, ncores, npc):
    """Per-core compaction of pooling sources.

    For each core, the union of edge sources feeding its nodes is laid
    out [pos-only | shared | neg-only] and padded to kt*128 rows.  The
    pos accumulation then only needs K-tiles [0, pos_k1), the neg one
    [neg_k0, kt).  Returns (kt, pass_ranges, per-core list of
    (src_rows, a_compact [2*npc, kt*128])).
    """
    src, lab, dst = edges[:, 0], edges[:, 1], edges[:, 2]
    per_core = []
    for c in range(ncores):
        lo, hi = c * npc, (c + 1) * npc
        # (dst_local, src_global) contribution pairs per sign
        pairs = []
        for mask in (lab > 0, lab < 0):
            s, d = src[mask], dst[mask]
            sel1 = (d >= lo) & (d < hi)
            sel2 = (s >= lo) & (s < hi)
            pairs.append((np.concatenate([d[sel1] - lo, s[sel2] - lo]),
                          np.concatenate([s[sel1], d[sel2]])))
        pos_srcs = set(pairs[0][1].tolist())
        neg_srcs = set(pairs[1][1].tolist())
        shared = pos_srcs & neg_srcs
        order = (sorted(pos_srcs - shared) + sorted(shared)
                 + sorted(neg_srcs - shared))
        per_core.append((pairs, len(pos_srcs - shared), len(shared),
                         len(neg_srcs - shared), order))

    kt = max(int(np.ceil((p + s + n) / 128)) for _, p, s, n, _ in per_core)
    pos_k1 = max(int(np.ceil((p + s) / 128)) for _, p, s, _, _ in per_core)
    neg_k0 = min(p // 128 for _, p, _, _, _ in per_core)
    pass_ranges = ((0, pos_k1), (neg_k0, kt))

    out = []
    for pairs, p, s, n, order in per_core:
        colof = {g: i for i, g in enumerate(order)}
        a = np.zeros((2 * npc, kt * 128), np.float32)
        for mi, (dl, sg) in enumerate(pairs):
            np.add.at(a, (mi * npc + dl, [colof[g] for g in sg]), 1.0)
        rows = np.zeros(kt * 128, np.int64)
        rows[:len(order)] = order
        out.append((rows, a))
    return kt, pass_ranges, out


@functools.lru_cache(maxsize=2)
def _build_module(npc, v, ncores, kt, pass_ranges):
    nc = bacc.Bacc(
        "TRN2", target_bir_lowering=False, debug=False,
        enable_asserts=False, num_devices=ncores,
    )
    aps = {
        "feats_pool": nc.dram_tensor("feats_pool", [(CHW // 512) * 128,
                                     kt * 512], BF16,
                                     kind="ExternalInput").ap(),
        "feats_shard": nc.dram_tensor("feats_shard", [npc, CHW], BF16,
                                      kind="ExternalInput").ap(),
        "a_lhsT": nc.dram_tensor("a_lhsT", [128, kt * 2 * npc], BF16,
                                 kind="ExternalInput").ap(),
        "wconv": nc.dram_tensor("wconv", [2 * C3, WCOLS], BF16,
                                kind="ExternalInput").ap(),
        "biases": nc.dram_tensor("biases", [128, 6], F32,
                                 kind="ExternalInput").ap(),
        "out": nc.dram_tensor("out", [npc, CHW], F32,
                              kind="ExternalOutput").ap(),
    }
    with tile.TileContext(nc) as tc:
        build_kernel(tc, aps, npc, v, kt, pass_ranges)
    nc.compile()
    return nc


def make_in_maps(feats, edges, w1a, b1a, w1b, b1b, w2a, b2a, w2b, b2b,
                 wf, bf, ncores=NCORES, v=V):
    import ml_dtypes
    feats = np.ascontiguousarray(np.asarray(feats, np.float32)).reshape(v, CHW)
    edges = np.asarray(edges)
    npc = v // ncores
    kt, pass_ranges, compact = _build_compact_pool(edges, v, ncores, npc)
    wsb = _prep_weights(
        [np.asarray(w) for w in (w1a, w1b, w2a, w2b, wf)],
        [np.asarray(b) for b in (b1a, b1b, b2a, b2b, bf)],
    )
    bias = _prep_biases(np.asarray(b1a), np.asarray(b2a), np.asarray(bf),
                    np.asarray(b1b), np.asarray(b2b))
    feats_bf = feats.astype(ml_dtypes.bfloat16)
    nch = CHW // 512
    in_maps = []
    for i in range(ncores):
        rows = slice(i * npc, (i + 1) * npc)
        src_rows, a_cmp = compact[i]
        fsel = feats_bf[src_rows]                      # [kt*128, CHW]
        fp = fsel.reshape(kt, 128, nch, 512).transpose(2, 1, 0, 3)
        fp = np.ascontiguousarray(fp).reshape(nch * 128, kt * 512)
        alt = a_cmp.T.reshape(kt, 128, 2 * npc).transpose(1, 0, 2)
        alt = np.ascontiguousarray(alt).reshape(128, kt * 2 * npc)
        in_maps.append({
            "feats_pool": fp,
            "feats_shard": np.ascontiguousarray(feats_bf[rows]),
            "a_lhsT": alt.astype(ml_dtypes.bfloat16),
            "wconv": wsb,
            "biases": bias,
        })
    return in_maps, kt, pass_ranges


def run(inputs, trace=False):
    in_maps, kt, pass_ranges = make_in_maps(**inputs)
    nc = _build_module(NPC, V, NCORES, kt, pass_ranges)
    res = bass_utils.run_bass_kernel_spmd(
        nc, in_maps, core_ids=list(range(NCORES)), trace=trace,
    )
    out = np.concatenate(
        [res.results[i]["out"] for i in range(NCORES)], axis=0
    ).reshape(V, C, H, H)
    return out, res


def kernel(**inputs):
    out, _ = run(inputs, trace=False)
    return out

